# revision 17
# baseline (speedup 1.0000x reference)
"""Trainium2 Bass kernel for causal GQA self-attention (B=4, T=2048, D=2048,
H=16, KVH=4, HD=128, partial RoPE 64, per-head RMS norm on q/k, xsa postproc,
out projection).

Sharding: 8 cores = (batch b = core//2) x (head-half = core%2).  Each core
handles one batch and 8 query heads (2 kv heads).  Core outputs a partial
out-projection (its 1024 feature columns of y); host sums the two partials
per batch.

All device math in fp32 (matmuls via float32r fast path).  Host pre-
transposes x and the weights so no fp32 DMA-transposes are needed on device.
"""

import sys

sys.path.insert(0, "/opt/trn_rl_repo")

from contextlib import ExitStack

import numpy as np

import concourse.bass as bass
import concourse.mybir as mybir
import concourse.tile as tile
from concourse import bacc
from concourse.bass_utils import run_bass_kernel_spmd
from concourse.masks import make_identity

FP32 = mybir.dt.float32
FP32R = mybir.dt.float32r

T = 2048
D = 2048
HD = 128
RD = 64
HALF = RD // 2  # 32
NQH = 8  # query heads per core
NKV = 2  # kv heads per core
REP = 4  # query heads per kv head
NG = 2  # head groups per core (one kv head + 4 q heads each)
SCALE = float(HD) ** -0.5
EPS_RMS = 1e-6
EPS_NORM = 1e-12

QT = 512  # query tile (free dim of ST tiles)
KT = 128  # key tile (partition dim of ST tiles)
NQT = T // QT  # 4
NDS = D // 128  # 16 contraction subtiles


def r(ap):
    """View an fp32 AP as float32r for fast PE matmuls (no-op if already)."""
    if ap.dtype == FP32R:
        return ap
    return ap.bitcast(FP32R)


def _proj_group(nc, tc, ctx, g, xT_d, wq_d, wk_d, wv_d, cs_sb, qg_sb,
                qTn, kTn, vT, vq, rnv2, ident, ones_sb, eps_sb):
    """Project x -> q (4 heads), k, v for head-group g; fused RMS norm for
    q/k; v transposed to [t, d]; 1/||v||^2 row computed."""
    es = ExitStack()
    with es:
        wpool = es.enter_context(tc.tile_pool(name=f"w{g}", bufs=1))
        xpool = es.enter_context(tc.tile_pool(name=f"x{g}", bufs=2))
        tpool = es.enter_context(tc.tile_pool(name=f"pt{g}", bufs=2))
        ropool = es.enter_context(tc.tile_pool(name=f"ro{g}", bufs=2))
        rpool = es.enter_context(tc.tile_pool(name=f"pr{g}", bufs=2))
        ppool = es.enter_context(
            tc.tile_pool(name=f"pp{g}", bufs=1, space="PSUM"))
        npool = es.enter_context(
            tc.tile_pool(name=f"pn{g}", bufs=1, space="PSUM"))

        # resident weights for this group
        wq_sb = wpool.tile([128, NDS, 512], FP32R)
        nc.sync.dma_start(wq_sb[:], wq_d[:, :, g * 512:(g + 1) * 512])
        wk_sb = wpool.tile([128, NDS, 128], FP32R)
        nc.sync.dma_start(wk_sb[:], wk_d[:, :, g * 128:(g + 1) * 128])
        wv_sb = wpool.tile([128, NDS, 128], FP32R)
        nc.sync.dma_start(wv_sb[:], wv_d[:, :, g * 128:(g + 1) * 128])

        for tt in range(NQT):
            tsl = slice(tt * QT, (tt + 1) * QT)
            # sin rows live at partitions 32-63 of cs_sb; DVE ops need all
            # operands on the same partitions, so stage them at 0-31
            sin_lo = ropool.tile([HALF, QT], FP32, tag="sinlo")
            nc.sync.dma_start(sin_lo[:], cs_sb[HALF:RD, tsl])
            # 6 PSUM accumulators: 4 q heads, k, v
            acc = [ppool.tile([128, QT], FP32, tag=f"acc{s}",
                              name=f"acc{s}")
                   for s in range(6)]
            for ds in range(NDS):
                xt = xpool.tile([128, QT], FP32R, tag="xt")
                nc.sync.dma_start(xt[:], xT_d[:, ds, tsl])
                for s in range(6):
                    if s < 4:
                        w = wq_sb[:, ds, s * 128:(s + 1) * 128]
                    elif s == 4:
                        w = wk_sb[:, ds, :]
                    else:
                        w = wv_sb[:, ds, :]
                    nc.tensor.matmul(acc[s][:], r(w), r(xt[:]),
                                     start=(ds == 0), stop=(ds == NDS - 1))

            # q/k: fused RMS norm (norm over d = partition dim)
            for s in range(5):
                sq = tpool.tile([128, QT], FP32R, tag="sq")
                nc.scalar.square(sq[:], acc[s][:])
                nrm = npool.tile([1, QT], FP32, tag="nrm")
                nc.tensor.matmul(nrm[:], r(ones_sb[:]), r(sq[:]),
                                 start=True, stop=True)
                # rn = 1/sqrt(mean + eps)  (mean = nrm/128)
                rn = rpool.tile([1, QT], FP32, tag="rn")
                nc.scalar.activation(rn[:], nrm[:],
                                     mybir.ActivationFunctionType.Sqrt,
                                     bias=eps_sb[0:1, 0:1], scale=1.0 / HD)
                nc.vector.reciprocal(rn[:], rn[:])
                if s < 4:
                    # fold per-head q_gain into rn
                    nc.vector.tensor_scalar_mul(
                        rn[:], rn[:], qg_sb[0:1, g * 4 + s:g * 4 + s + 1])
                rnb = rpool.tile([128, QT], FP32, tag="rnb")
                nc.gpsimd.partition_broadcast(rnb[:], rn[:])
                dst = qTn[:, s, tsl] if s < 4 else kTn[:, tsl]
                nc.vector.tensor_mul(out=dst, in0=acc[s][:], in1=rnb[:])

                # partial RoPE on the freshly normed tile (rotates partition
                # pairs (i, i+32) of the first 64 dims); all math staged on
                # partitions 0-31, high half moved via SBUF-SBUF DMA
                x1 = dst[0:HALF, :]
                cosr = cs_sb[0:HALF, tsl]
                x2lo = ropool.tile([HALF, QT], FP32R, tag="x2lo")
                nc.sync.dma_start(x2lo[:], dst[HALF:RD, :])
                ta = ropool.tile([HALF, QT], FP32, tag="ropea")
                tb = ropool.tile([HALF, QT], FP32, tag="ropeb")
                td = ropool.tile([HALF, QT], FP32, tag="roped")
                nc.vector.tensor_mul(out=ta[:], in0=x1, in1=cosr)
                nc.vector.tensor_mul(out=tb[:], in0=x2lo[:], in1=sin_lo[:])
                nc.vector.tensor_mul(out=td[:], in0=x1, in1=sin_lo[:])
                nc.vector.tensor_sub(out=x1, in0=ta[:], in1=tb[:])
                hi = ropool.tile([HALF, QT], FP32R, tag="ropehi")
                nc.vector.tensor_mul(out=ta[:], in0=x2lo[:], in1=cosr)
                nc.vector.tensor_add(out=hi[:], in0=ta[:], in1=td[:])
                nc.sync.dma_start(dst[HALF:RD, :], hi[:])

            # v: copy out, plus ||v||^-2 row and [t, d] transpose
            nc.scalar.copy(vT[:, tsl], acc[5][:])
            sqv = tpool.tile([128, QT], FP32R, tag="sq")
            nc.scalar.square(sqv[:], acc[5][:])
            nv = npool.tile([1, QT], FP32, tag="nrm")
            nc.tensor.matmul(nv[:], r(ones_sb[:]), r(sqv[:]),
                             start=True, stop=True)
            rv = rpool.tile([1, QT], FP32, tag="rn")
            nc.scalar.sqrt(rv[:], nv[:])  # ||v||
            nc.vector.tensor_scalar_max(rv[:], rv[:], EPS_NORM)
            nc.vector.reciprocal(rv[:], rv[:])  # 1/||v||
            nc.vector.tensor_mul(out=rnv2[0:1, tsl], in0=rv[:], in1=rv[:])

        # v transpose to [t, d] via PE (after vT fully written)
        vtp = es.enter_context(
            tc.tile_pool(name=f"pvt{g}", bufs=1, space="PSUM"))
        for ts128 in range(T // 128):
            pvt = vtp.tile([128, 128], FP32, tag="pvt")
            nc.tensor.transpose(pvt[:], vT[:, ts128 * 128:(ts128 + 1) * 128],
                                ident[:])
            nc.scalar.copy(vq[:, ts128, :], pvt[:])




def _attn_group(nc, tc, ctx, g, qTn, kTn, vT, vq, rnv2, ones_sb, yT):
    """Causal attention + xsa postproc for the 4 query heads of group g.
    Writes normalized, xsa-corrected yT [128(d), 4(head), T]."""
    es = ExitStack()
    with es:
        ptp = es.enter_context(tc.tile_pool(name=f"ptp{g}", bufs=4))
        vp = es.enter_context(tc.tile_pool(name=f"vp{g}", bufs=2))
        rp = es.enter_context(tc.tile_pool(name=f"rp{g}", bufs=2))
        pst = es.enter_context(
            tc.tile_pool(name=f"pst{g}", bufs=2, space="PSUM"))
        py = es.enter_context(
            tc.tile_pool(name=f"py{g}", bufs=2, space="PSUM"))
        pden = es.enter_context(
            tc.tile_pool(name=f"pden{g}", bufs=2, space="PSUM"))
        pdot = es.enter_context(
            tc.tile_pool(name=f"pdot{g}", bufs=2, space="PSUM"))

        for h in range(4):
            qh = qTn[:, h, :]
            for qi in range(NQT):
                q0 = qi * QT
                qsl = slice(q0, q0 + QT)
                n_kt = (q0 + QT) // KT
                psum_y = py.tile([128, QT], FP32, tag="y")
                psum_den = pden.tile([1, QT], FP32, tag="den")
                for kt in range(n_kt):
                    st = pst.tile([128, QT], FP32, tag="st")
                    nc.tensor.matmul(st[:],
                                     r(kTn[:, kt * KT:(kt + 1) * KT]),
                                     r(qh[:, qsl]), start=True, stop=True)
                    pt = ptp.tile([128, QT], FP32R, tag="pt")
                    nc.scalar.activation(pt[:], st[:],
                                         mybir.ActivationFunctionType.Exp,
                                         scale=SCALE)
                    if kt >= n_kt - 4:
                        # diagonal block: zero out k > q (post-exp)
                        nc.gpsimd.affine_select(
                            out=pt[:], in_=pt[:],
                            compare_op=mybir.AluOpType.is_ge,
                            fill=0.0, base=q0 - kt * KT,
                            channel_multiplier=-1, pattern=[[1, QT]])
                    nc.tensor.matmul(psum_den[:], r(ones_sb[:]), r(pt[:]),
                                     start=(kt == 0), stop=(kt == n_kt - 1))
                    nc.tensor.matmul(psum_y[:], r(vq[:, kt, :]), r(pt[:]),
                                     start=(kt == 0), stop=(kt == n_kt - 1))

                rden = rp.tile([1, QT], FP32, tag="rden")
                nc.vector.reciprocal(rden[:], psum_den[:])
                rnb = rp.tile([128, QT], FP32, tag="rnb")
                nc.gpsimd.partition_broadcast(rnb[:], rden[:])
                yn = vp.tile([128, QT], FP32, tag="yn")
                nc.vector.tensor_mul(out=yn[:], in0=psum_y[:], in1=rnb[:])

                # xsa: y -= (y . v) * v / ||v||^2   (per token column)
                w1 = vp.tile([128, QT], FP32R, tag="w1")
                nc.vector.tensor_mul(out=w1[:], in0=yn[:], in1=vT[:, qsl])
                pd = pdot.tile([1, QT], FP32, tag="dot")
                nc.tensor.matmul(pd[:], r(ones_sb[:]), r(w1[:]),
                                 start=True, stop=True)
                dot2 = rp.tile([1, QT], FP32, tag="dot2")
                nc.vector.tensor_mul(out=dot2[:], in0=pd[:],
                                     in1=rnv2[0:1, qsl])
                dotb = rp.tile([128, QT], FP32, tag="dotb")
                nc.gpsimd.partition_broadcast(dotb[:], dot2[:])
                w2 = vp.tile([128, QT], FP32, tag="w2")
                nc.vector.tensor_mul(out=w2[:], in0=dotb[:], in1=vT[:, qsl])
                nc.vector.tensor_sub(out=yT[:, h, qsl], in0=yn[:], in1=w2[:])


def _outproj_group(nc, tc, ctx, g, yT, wp_d, out_d):
    """Partial out-projection: out_d[g] = (Wproj[:, fslice] y)  as [o, t]."""
    es = ExitStack()
    with es:
        wpp = es.enter_context(tc.tile_pool(name=f"wpp{g}", bufs=2))
        op = es.enter_context(tc.tile_pool(name=f"op{g}", bufs=3))
        pop = es.enter_context(
            tc.tile_pool(name=f"pop{g}", bufs=2, space="PSUM"))
        for os_ in range(16):
            wp_sb = wpp.tile([128, 4, 128], FP32R, tag="wp")
            nc.sync.dma_start(
                wp_sb[:], wp_d[:, g * 4:(g + 1) * 4,
                               os_ * 128:(os_ + 1) * 128])
            for tt in range(NQT):
                tsl = slice(tt * QT, (tt + 1) * QT)
                po = pop.tile([128, QT], FP32, tag="po")
                for fs in range(4):
                    nc.tensor.matmul(po[:], r(wp_sb[:, fs, :]),
                                     r(yT[:, fs, tsl]),
                                     start=(fs == 0), stop=(fs == 3))
                ot = op.tile([128, QT], FP32, tag="ot")
                nc.scalar.copy(ot[:], po[:])
                nc.sync.dma_start(out_d[g, :, os_, tsl], ot[:])


def build_program():
    nc = bacc.Bacc(None)

    xT_d = nc.dram_tensor("xT", [D, T], FP32R, kind="ExternalInput")
    wq_d = nc.dram_tensor("wqT", [D, NQH * HD], FP32R, kind="ExternalInput")
    wk_d = nc.dram_tensor("wkT", [D, NKV * HD], FP32R, kind="ExternalInput")
    wv_d = nc.dram_tensor("wvT", [D, NKV * HD], FP32R, kind="ExternalInput")
    wp_d = nc.dram_tensor("wpT", [NQH * HD, D], FP32R, kind="ExternalInput")
    cs_d = nc.dram_tensor("cs", [RD, T], FP32, kind="ExternalInput")
    qg_d = nc.dram_tensor("qg", [1, NQH], FP32, kind="ExternalInput")
    out_d = nc.dram_tensor("outT", [NG, D, T], FP32, kind="ExternalOutput")

    xT = xT_d[:].rearrange("(po pi) t -> pi po t", pi=128)
    wq = wq_d[:].rearrange("(po pi) m -> pi po m", pi=128)
    wk = wk_d[:].rearrange("(po pi) m -> pi po m", pi=128)
    wv = wv_d[:].rearrange("(po pi) m -> pi po m", pi=128)
    wp = wp_d[:].rearrange("(fo fi) o -> fi fo o", fi=128)
    out = out_d[:].rearrange("g (oo oi) t -> g oi oo t", oi=128)

    with tile.TileContext(nc) as tc:
        ctx = ExitStack()
        with ctx:
            cpool = ctx.enter_context(tc.tile_pool(name="const", bufs=1))
            ident = cpool.tile([128, 128], FP32)
            make_identity(nc, ident[:])
            ones_f = cpool.tile([128, 1], FP32)
            nc.vector.memset(ones_f[:], 1.0)
            ones_sb = cpool.tile([128, 1], FP32R)
            nc.scalar.copy(ones_sb[:], ones_f[:])
            cs_sb = cpool.tile([RD, T], FP32)
            nc.sync.dma_start(cs_sb[:], cs_d[:])
            qg_sb = cpool.tile([1, NQH], FP32)
            nc.sync.dma_start(qg_sb[:], qg_d[:])
            eps_sb = cpool.tile([1, 1], FP32)
            nc.vector.memset(eps_sb[:], EPS_RMS)

            for g in range(NG):
                ges = ExitStack()
                with ges:
                    gp = ges.enter_context(
                        tc.tile_pool(name=f"grp{g}", bufs=1))
                    qTn = gp.tile([128, 4, T], FP32R)
                    kTn = gp.tile([128, T], FP32R)
                    vT = gp.tile([128, T], FP32)
                    vq = gp.tile([128, T // 128, 128], FP32R)
                    rnv2 = gp.tile([1, T], FP32)
                    yT = gp.tile([128, 4, T], FP32R)

                    _proj_group(nc, tc, ctx, g, xT, wq, wk, wv, cs_sb,
                                qg_sb, qTn, kTn, vT, vq, rnv2, ident,
                                ones_sb, eps_sb)
                    _attn_group(nc, tc, ctx, g, qTn, kTn, vT, vq, rnv2,
                                ones_sb, yT)
                    _outproj_group(nc, tc, ctx, g, yT, wp, out)
    nc.finalize()
    return nc


_NC_CACHE = None


def _get_nc():
    global _NC_CACHE
    if _NC_CACHE is None:
        _NC_CACHE = build_program()
    return _NC_CACHE


def _rope_cache_T():
    inv = (1.0 / (10000.0 ** (np.arange(0, RD, 2, dtype=np.float32)
                              / np.float32(RD)))).astype(np.float32)
    t = np.arange(T, dtype=np.float32)
    freqs = (t[:, None] * inv[None, :]).astype(np.float32)  # [T, 32]
    return np.ascontiguousarray(
        np.concatenate([np.cos(freqs).T, np.sin(freqs).T], axis=0)
    ).astype(np.float32)  # [64, T]


def kernel(x, Wq, Wk, Wv, Wproj, q_gain, **_ignored):
    x = np.asarray(x, dtype=np.float32)
    Wq = np.asarray(Wq, dtype=np.float32)
    Wk = np.asarray(Wk, dtype=np.float32)
    Wv = np.asarray(Wv, dtype=np.float32)
    Wproj = np.asarray(Wproj, dtype=np.float32)
    q_gain = np.asarray(q_gain, dtype=np.float32)

    cs = _rope_cache_T()
    in_maps = []
    for c in range(8):
        b, half = c // 2, c % 2
        fsl = slice(half * 1024, (half + 1) * 1024)
        in_maps.append({
            "xT": np.ascontiguousarray(x[b].T),
            "wqT": np.ascontiguousarray(Wq[fsl].T),
            "wkT": np.ascontiguousarray(Wk[half * 256:(half + 1) * 256].T),
            "wvT": np.ascontiguousarray(Wv[half * 256:(half + 1) * 256].T),
            "wpT": np.ascontiguousarray(Wproj[:, fsl].T),
            "cs": cs,
            "qg": np.ascontiguousarray(
                q_gain[half * 8:(half + 1) * 8].reshape(1, 8)),
        })

    nc = _get_nc()
    res = run_bass_kernel_spmd(nc, in_maps, core_ids=list(range(8)))

    out = np.zeros((4, T, D), np.float32)
    for c in range(8):
        o = res.results[c]["outT"]  # [2, D(o), T]
        out[c // 2] += (o[0] + o[1]).T
    return out
